# revision 5
# baseline (speedup 1.0000x reference)
"""Trainium2 Bass kernel for nn_MetapathEmbed.

Math:  out[b,m] = sum_{c,d,e} pools[b,d] * metapath[c,d] * card[c,e] * kern[e,m]
Factored:
    X = card @ kern                  [C, M]
    W[d,m] = sum_c metapath[c,d] * X[c,m]
    out = pools @ W                  [B, M]

Sharding: metapath / card row-sharded over c across 8 cores; each core
computes its W_partial and out_partial = pools @ W_partial [B, M]; the host
sums the 8 partials (associativity of the c-sum).

Precision scheme: the hardware PE runs fp32 matmuls as 2 half-rate passes
(4 cycles/column). Instead, every matmul operand is split into bf16
hi + lo (a.k.a. "split-float": x = bf16(x) + bf16(x - bf16(x))) and each
product is computed with 3 full-rate bf16 matmuls accumulated in fp32 PSUM
(hi*hi + lo*hi + hi*lo), giving ~2^-17 relative accuracy at 3/4 the PE
cost of fp32.  metapath and poolsT are split on the host (pure input
re-encoding, same total bytes: 2+2 vs 4 per element); X and W are split
on-chip.

Per-core dataflow:
  - X[c,m] from host-pre-transposed cardT; split to Xh/Xl bf16.
  - Main loop over 8 d-groups of 2048: W^T[m,d] accumulated in 4 PSUM
    banks over 16 c-chunks; stationary Xh/Xl, moving metapath hi/lo tiles
    (N=512 per matmul). One 1-MiB DMA per (group, c-chunk), alternating
    between the two HWDGE rings.
  - Per group: evacuate W^T to SBUF, TensorE-transpose 128x128 tiles to
    W[d,m], split to Wh/Wl, and fold into out_psum[b,m] with 3-term bf16
    matmuls against host-pre-transposed poolsT hi/lo.
"""

import sys

if "/opt/trn_rl_repo" not in sys.path:
    sys.path.insert(0, "/opt/trn_rl_repo")

import numpy as np

B, C, E, M = 128, 16384, 64, 128
N_CORES = 8
R = C // N_CORES          # 2048 metapath rows per core
RC = R // 128             # 16 c-chunks per core
DG = 2048                 # d-group width (4 psum banks of 512)
NG = C // DG              # 8 d-groups
TPG = DG // 128           # 16 transpose tiles per group
ND = C // 128             # 128 d-chunks total (final contraction)

_NC_CACHE = {}


def _build_nc():
    if "nc" in _NC_CACHE:
        return _NC_CACHE["nc"]

    from concourse import bacc, mybir
    from concourse.tile import TileContext
    from concourse.masks import make_identity

    dt = mybir.dt
    nc = bacc.Bacc(None, target_bir_lowering=False)

    # metapath hi/lo interleaved: [R, g, {hi,lo}, DG] flattened
    mp_d = nc.declare_dram_parameter("mp", [R, 2 * C], dt.bfloat16, isOutput=False)
    pth_d = nc.declare_dram_parameter("pth", [128, C], dt.bfloat16, isOutput=False)
    ptl_d = nc.declare_dram_parameter("ptl", [128, C], dt.bfloat16, isOutput=False)
    cardT_d = nc.declare_dram_parameter("cardT", [E, R], dt.float32, isOutput=False)
    kern_d = nc.declare_dram_parameter("kern", [E, M], dt.float32, isOutput=False)
    out_d = nc.declare_dram_parameter("out", [B, M], dt.float32, isOutput=True)

    with TileContext(nc) as tc:
        with (
            tc.tile_pool(name="const", bufs=1) as const_pool,
            tc.tile_pool(name="big", bufs=1) as big_pool,
            tc.tile_pool(name="mp", bufs=3) as mp_pool,
            tc.tile_pool(name="stage", bufs=2) as stage_pool,
            tc.tile_pool(name="psA", bufs=1, space="PSUM") as ps_a,
            tc.tile_pool(name="psB", bufs=1, space="PSUM") as ps_b,
            tc.tile_pool(name="psC", bufs=1, space="PSUM") as ps_c,
            tc.tile_pool(name="psD", bufs=1, space="PSUM") as ps_d,
            tc.tile_pool(name="psT", bufs=2, space="PSUM") as psum_t,
            tc.tile_pool(name="psO", bufs=1, space="PSUM") as psum_o,
        ):
            # ---- Phase A: constants + X = card @ kern, split hi/lo ----
            ident = const_pool.tile([128, 128], dt.float32)
            make_identity(nc, ident)

            kern_sb = const_pool.tile([E, M], dt.float32)
            nc.gpsimd.dma_start(out=kern_sb, in_=kern_d[:, :])
            cardT_sb = const_pool.tile([E, R], dt.float32)
            nc.gpsimd.dma_start(out=cardT_sb, in_=cardT_d[:, :])
            pth_sb = big_pool.tile([128, C], dt.bfloat16)  # [d_in, d_out*B + b]
            nc.gpsimd.dma_start(out=pth_sb, in_=pth_d[:, :])
            ptl_sb = big_pool.tile([128, C], dt.bfloat16)
            nc.gpsimd.dma_start(out=ptl_sb, in_=ptl_d[:, :])

            xh_sb = big_pool.tile([128, RC * M], dt.bfloat16)  # [c_in, chunk*M + m]
            xl_sb = big_pool.tile([128, RC * M], dt.bfloat16)
            for k in range(RC):
                psx = psum_t.tile([128, M], dt.float32, tag="pst")
                nc.tensor.matmul(
                    psx,
                    cardT_sb[:, k * 128 : (k + 1) * 128],  # [e, c_in]
                    kern_sb[:, :],                          # [e, m]
                    start=True,
                    stop=True,
                )
                xh = xh_sb[:, k * M : (k + 1) * M]
                xl = xl_sb[:, k * M : (k + 1) * M]
                nc.vector.tensor_copy(out=xh, in_=psx)        # cast fp32->bf16
                nc.vector.tensor_sub(out=xl, in0=psx, in1=xh)  # residual

            # ---- Phases B+C: main accumulation + per-group finalization ----
            wh_sb = big_pool.tile([128, ND * M], dt.bfloat16)  # [d_in, d_out*M + m]
            wl_sb = big_pool.tile([128, ND * M], dt.bfloat16)
            out_ps = psum_o.tile([B, M], dt.float32)

            for g in range(NG):
                pss = [
                    p.tile([128, 512], dt.float32, tag=f"ps{j}", name=f"ps{j}")
                    for j, p in enumerate([ps_a, ps_b, ps_c, ps_d])
                ]
                for ci in range(RC):
                    mp_t = mp_pool.tile([128, 2 * DG], dt.bfloat16)
                    dma_eng = nc.sync if ci % 2 == 0 else nc.scalar
                    dma_eng.dma_start(
                        out=mp_t,
                        in_=mp_d[
                            ci * 128 : (ci + 1) * 128, g * 2 * DG : (g + 1) * 2 * DG
                        ],
                    )
                    hi = mp_t[:, 0:DG]
                    lo = mp_t[:, DG : 2 * DG]
                    xh = xh_sb[:, ci * M : (ci + 1) * M]
                    xl = xl_sb[:, ci * M : (ci + 1) * M]
                    first = ci == 0
                    last = ci == RC - 1
                    # stationary Xh: hi then lo; stationary Xl: hi.
                    for k in range(4):
                        nc.tensor.matmul(
                            pss[k], xh, hi[:, k * 512 : (k + 1) * 512],
                            start=first, stop=False,
                        )
                    for k in range(4):
                        nc.tensor.matmul(
                            pss[k], xh, lo[:, k * 512 : (k + 1) * 512],
                            start=False, stop=False,
                        )
                    for k in range(4):
                        nc.tensor.matmul(
                            pss[k], xl, hi[:, k * 512 : (k + 1) * 512],
                            start=False, stop=last,
                        )

                # evacuate W^T group to SBUF staging
                stage = stage_pool.tile([128, DG], dt.float32)  # [m, d_local]
                for k in range(4):
                    nc.vector.tensor_copy(
                        out=stage[:, k * 512 : (k + 1) * 512], in_=pss[k]
                    )

                # transpose to W layout, split hi/lo, fold into out_psum
                for t in range(TPG):
                    d_out = g * TPG + t
                    pst = psum_t.tile([128, 128], dt.float32, tag="pst")
                    nc.tensor.transpose(
                        pst, stage[:, t * 128 : (t + 1) * 128], ident
                    )
                    wh = wh_sb[:, d_out * M : (d_out + 1) * M]
                    wl = wl_sb[:, d_out * M : (d_out + 1) * M]
                    nc.vector.tensor_copy(out=wh, in_=pst)
                    nc.vector.tensor_sub(out=wl, in0=pst, in1=wh)
                    pth_t = pth_sb[:, d_out * 128 : (d_out + 1) * 128]  # [d_in, b]
                    ptl_t = ptl_sb[:, d_out * 128 : (d_out + 1) * 128]
                    first = d_out == 0
                    last = d_out == ND - 1
                    # stationary PTh: Wh then Wl; stationary PTl: Wh.
                    nc.tensor.matmul(
                        out_ps, pth_t, wh,
                        start=first, stop=False, skip_group_check=True,
                    )
                    nc.tensor.matmul(
                        out_ps, pth_t, wl,
                        start=False, stop=False, skip_group_check=True,
                    )
                    nc.tensor.matmul(
                        out_ps, ptl_t, wh,
                        start=False, stop=last, skip_group_check=True,
                    )

            out_sb = const_pool.tile([B, M], dt.float32)
            nc.vector.tensor_copy(out=out_sb, in_=out_ps)
            nc.sync.dma_start(out=out_d[:, :], in_=out_sb)

    nc.compile()
    _NC_CACHE["nc"] = nc
    return nc


def _split_hi_lo(a):
    import ml_dtypes

    hi = a.astype(ml_dtypes.bfloat16)
    lo = (a - hi.astype(np.float32)).astype(ml_dtypes.bfloat16)
    return hi, lo


def _prep_in_maps(batch_pools, metapath, card_embeddings, kern):
    batch_pools = np.ascontiguousarray(batch_pools, dtype=np.float32)
    metapath = np.ascontiguousarray(metapath, dtype=np.float32)
    card_embeddings = np.ascontiguousarray(card_embeddings, dtype=np.float32)
    kern = np.ascontiguousarray(kern, dtype=np.float32)

    # poolsT tiled: [d_in, d_out*B + b] = pools[b, d_out*128 + d_in]
    poolsT = batch_pools.T.reshape(128, 128, B).transpose(1, 0, 2).reshape(128, C)
    pth, ptl = _split_hi_lo(np.ascontiguousarray(poolsT))

    in_maps = []
    for i in range(N_CORES):
        sl = slice(i * R, (i + 1) * R)
        # metapath hi/lo interleaved per d-group: [R, g, {hi,lo}, DG]
        mslice = metapath[sl].reshape(R, NG, DG)
        hi, lo = _split_hi_lo(mslice)
        mp_hl = np.ascontiguousarray(
            np.stack([hi, lo], axis=2).reshape(R, 2 * C)
        )
        card_slice = card_embeddings[sl]
        cardT = np.ascontiguousarray(
            card_slice.reshape(RC, 128, E).transpose(2, 0, 1).reshape(E, R)
        )
        in_maps.append(
            {"mp": mp_hl, "pth": pth, "ptl": ptl, "cardT": cardT, "kern": kern}
        )
    return in_maps


def _run(inputs, **spmd_kwargs):
    from concourse.bass_utils import run_bass_kernel_spmd

    nc = _build_nc()
    in_maps = _prep_in_maps(
        inputs["batch_pools"],
        inputs["metapath"],
        inputs["card_embeddings"],
        inputs["kernel"],
    )
    res = run_bass_kernel_spmd(nc, in_maps, core_ids=list(range(N_CORES)), **spmd_kwargs)
    acc = np.zeros((B, M), dtype=np.float64)
    for r in res.results:
        acc += r["out"].astype(np.float64)
    return acc.astype(np.float32), res


def kernel(**inputs):
    out, _ = _run(inputs)
    return out
